# revision 1
# baseline (speedup 1.0000x reference)
"""Per-sample depthwise 7x7 SAME cross-correlation on 8 trn2 NeuronCores.

Problem: inputs [32,128,128,128] (B,H,W,C), kernels [32,7,7,128] (B,KH,KW,C).
out[b,y,x,c] = sum_{i,j} inputs[b, y+i-3, x+j-3, c] * kernels[b,i,j,c]

Strategy (pure data parallel, batch sharded 4 samples/core):
  - Host: transpose to channel-major [b, c, y, x], zero-pad spatially to
    134x134 so every tap is a plain shifted AP read (SAME padding built in).
  - On-chip layout: C=128 on partitions, (y, x) in the free dim. The
    per-(b,c) kernel tap value is a per-partition scalar, so each tap is one
    fused multiply-accumulate: scalar_tensor_tensor(acc = x_shift * w + acc).
  - Taps are split across VectorE (fused MACs, 32 taps) and GpSimdE (adds of
    per-partition-scaled products that ScalarE produces, 17 taps), so all
    three elementwise-capable engines run concurrently; the two partial
    accumulators are merged on VectorE and DMA'd out channel-major.
  - Host transposes the gathered result back to [B,H,W,C].

Why not the TensorEngine: a depthwise conv with per-(b,c) kernels has no
shared contraction — any matmul formulation either needs per-channel banded
weight matrices (whose on-chip materialization costs more than the conv
itself: 3584 128x128 bands vs 512 images) or wastes >=127/128 of the array
on diagonal weights. The elementwise path on VectorE is the real roofline.
"""

import numpy as np

import concourse.bass as bass
import concourse.tile as tile
from concourse import bacc, mybir
from concourse.bass_utils import run_bass_kernel_spmd

B, H, W, C = 32, 128, 128, 128
KH = KW = 7
PAD = 3
N_CORES = 8
BPC = B // N_CORES  # samples per core
HP, WP = H + 2 * PAD, W + 2 * PAD  # 134, 134
SLAB = 32  # output rows per compute slab
N_SLABS = H // SLAB

# Tap split across the engines (tuned via cost-model + HW sweep).
_ALL_TAPS = [(i, j) for i in range(KH) for j in range(KW)]
N_GP_TAPS = 18
_GP_TAPS = _ALL_TAPS[:N_GP_TAPS]
_DVE_TAPS = _ALL_TAPS[N_GP_TAPS:]
# Independent VectorE accumulator chains: back-to-back dependent DVE ops pay
# a pipeline DRAIN ~= op duration (measured 2.15x); interleaved independent
# chains overlap it (measured 1.88x recovery on a DVE-only variant).
N_DVE_CHAINS = 3

_PROGRAM_CACHE = {}


def _build_program(repeat=1):
    f32 = mybir.dt.float32
    nc = bacc.Bacc("TRN2", target_bir_lowering=False, debug=False)
    x_h = nc.dram_tensor("x", [BPC, C, HP, WP], f32, kind="ExternalInput")
    w_h = nc.dram_tensor("w", [BPC, C, KH * KW], f32, kind="ExternalInput")
    o_h = nc.dram_tensor("o", [BPC, C, H, W], f32, kind="ExternalOutput")
    x, w, o = x_h.ap(), w_h.ap(), o_h.ap()

    with tile.TileContext(nc) as tc:
        with (
            tc.tile_pool(name="wpool", bufs=1) as wpool,
            tc.tile_pool(name="xpool", bufs=3) as xpool,
            tc.tile_pool(name="accd0", bufs=2) as accd0p,
            tc.tile_pool(name="accdx", bufs=1) as accdxp,
            tc.tile_pool(name="accg", bufs=2) as accgp,
            tc.tile_pool(name="accg1", bufs=1) as accg1p,
            tc.tile_pool(name="tmp", bufs=2) as tmpp,
        ):
            wall = wpool.tile([C, BPC, KH * KW], f32)
            for b in range(BPC):
                nc.sync.dma_start(out=wall[:, b, :], in_=w[b])

            for b, s in [
                (b, s)
                for _ in range(repeat)
                for b in range(BPC)
                for s in range(N_SLABS)
            ]:
                if True:
                    y0 = s * SLAB
                    xt = xpool.tile([C, SLAB + 2 * PAD, WP], f32)
                    nc.sync.dma_start(out=xt, in_=x[b, :, y0 : y0 + SLAB + 2 * PAD, :])

                    dacc = [
                        (accd0p if ch == 0 else accdxp).tile(
                            [C, SLAB, W], f32, name=f"dacc{ch}", tag=f"dacc{ch}"
                        )
                        for ch in range(N_DVE_CHAINS)
                    ]
                    if _GP_TAPS:
                        acc_g = accgp.tile([C, SLAB, W], f32)
                        acc_g1 = accg1p.tile([C, SLAB, W], f32)
                        gacc = [acc_g, acc_g1]
                    else:
                        acc_g = None

                    started = [False] * N_DVE_CHAINS
                    for t, (i, j) in enumerate(_DVE_TAPS):
                        ch = t % N_DVE_CHAINS
                        xin = xt[:, i : i + SLAB, j : j + W]
                        wsc = wall[:, b, i * KW + j : i * KW + j + 1]
                        if not started[ch]:
                            nc.vector.tensor_scalar_mul(dacc[ch], xin, wsc)
                            started[ch] = True
                        else:
                            nc.vector.scalar_tensor_tensor(
                                out=dacc[ch], in0=xin, scalar=wsc, in1=dacc[ch],
                                op0=mybir.AluOpType.mult, op1=mybir.AluOpType.add,
                            )
                    # GpSimd side: 2 interleaved accumulator chains (same
                    # drain-overlap rationale as the VectorE chains); ScalarE
                    # seeds each chain and produces every product.
                    gstarted = [False, False]
                    for t, (i, j) in enumerate(_GP_TAPS):
                        gch = t % 2
                        xin = xt[:, i : i + SLAB, j : j + W]
                        wsc = wall[:, b, i * KW + j : i * KW + j + 1]
                        if not gstarted[gch]:
                            nc.scalar.mul(gacc[gch], xin, wsc)
                            gstarted[gch] = True
                        else:
                            prod = tmpp.tile([C, SLAB, W], f32)
                            nc.scalar.mul(prod, xin, wsc)
                            nc.gpsimd.tensor_add(gacc[gch], gacc[gch], prod)
                    nc.gpsimd.tensor_add(acc_g, acc_g, acc_g1)
                    # Tree merge: first level is two INDEPENDENT adds whose
                    # pipeline drains overlap; only the final add is serial.
                    if N_DVE_CHAINS == 3 and acc_g is not None:
                        nc.vector.tensor_add(dacc[0], dacc[0], dacc[1])
                        nc.vector.tensor_add(dacc[2], dacc[2], acc_g)
                        nc.vector.tensor_add(dacc[0], dacc[0], dacc[2])
                    else:
                        for ch in range(1, N_DVE_CHAINS):
                            nc.vector.tensor_add(dacc[0], dacc[0], dacc[ch])
                        if acc_g is not None:
                            nc.vector.tensor_add(dacc[0], dacc[0], acc_g)
                    nc.sync.dma_start(out=o[b, :, y0 : y0 + SLAB, :], in_=dacc[0])

    nc.compile()
    return nc


def _get_program():
    if "nc" not in _PROGRAM_CACHE:
        _PROGRAM_CACHE["nc"] = _build_program()
    return _PROGRAM_CACHE["nc"]


def _prep_inputs(inputs, kernels):
    """Host-side shard + layout transform. Returns per-core input maps."""
    xt = _PROGRAM_CACHE.get("xt")
    if xt is None:
        xt = np.zeros((B, C, HP, WP), np.float32)
        _PROGRAM_CACHE["xt"] = xt
    xt[:, :, PAD : PAD + H, PAD : PAD + W] = np.transpose(inputs, (0, 3, 1, 2))
    wt = np.ascontiguousarray(
        np.transpose(kernels, (0, 3, 1, 2)).reshape(B, C, KH * KW)
    )
    in_maps = []
    for k in range(N_CORES):
        sl = slice(k * BPC, (k + 1) * BPC)
        in_maps.append({"x": xt[sl], "w": wt[sl]})
    return in_maps


def _gather_output(results):
    full = np.concatenate([r["o"] for r in results], axis=0)  # [B, C, H, W]
    return np.ascontiguousarray(np.transpose(full, (0, 2, 3, 1)))


def run_spmd(inputs, kernels, **spmd_kwargs):
    """Run on all 8 cores; returns (output, BassKernelResults)."""
    nc = _get_program()
    in_maps = _prep_inputs(np.asarray(inputs), np.asarray(kernels))
    res = run_bass_kernel_spmd(nc, in_maps, list(range(N_CORES)), **spmd_kwargs)
    return _gather_output(res.results), res


def kernel(inputs, kernels):
    out, _ = run_spmd(inputs, kernels)
    return out



# revision 4
# speedup vs baseline: 8.9927x; 8.9927x over previous
"""Per-sample depthwise 7x7 SAME cross-correlation on 8 trn2 NeuronCores.

Problem: inputs [32,128,128,128] (B,H,W,C), kernels [32,7,7,128] (B,KH,KW,C).
out[b,y,x,c] = sum_{i,j} inputs[b, y+i-3, x+j-3, c] * kernels[b,i,j,c]

Strategy (pure data parallel, 4 samples/core, TensorEngine formulation):
  For one (b, c) channel image X [y', x] the 2D conv factors as 7 banded
  matmuls accumulated in PSUM:

      out[y, x] = sum_j  sum_{y'}  T_j[y', y] * X[y', x + j - 3]
      T_j[y', y] = w[y' - y + 3, j]   (7-diagonal banded Toeplitz)

  lhsT (stationary) = T_j (K=y'=128, M=y=128), rhs (moving) = the image
  read at free-dim offset j from an x-padded SBUF tile, N = x = 128.
  One PSUM tile accumulates all 49 taps (7 matmuls x 7-wide band), so the
  PE does 7 fused MACs per streamed column-row vs 1/cycle/partition for
  any elementwise engine.

  The 3584 per-(b,c,j) banded matrices are NOT materialized densely: their
  zero cells never change, so each rebuild only rewrites the band. With
  the K axis flipped (host stores image rows reversed; the flip cancels in
  the contraction) the band cell (p, y) reads w[130 - p - y]: every
  8-column chunk of T is a rectangular [<=14, 8] block reading a tiny
  per-(b,c,j) DRAM buffer wr[u] = w[130-u] at u = p + y - an overlapping
  positive-stride affine AP. T tiles are c-minor ([128p, 128m, 7j, CBc])
  and wr is [u, j, c]-ordered so one chunk write for a whole channel-batch
  is one 3.5 KB-contiguous descriptor per partition on both sides. The
  first use of each double buffer writes the full partition range (zeros
  included, same buffers); steady state rewrites bands only.

  bf16 operands (PSUM accumulates fp32); rel err ~4e-3 << 2e-2 gate.
  Per core: 3584 matmuls ~= 191 us (cost model) vs 2.43 ms for the
  elementwise formulation. DMA ~= 130 us, hidden under PE.
"""

import numpy as np
import ml_dtypes

import concourse.bass as bass
import concourse.tile as tile
from concourse import bacc, mybir
from concourse.bass_utils import run_bass_kernel_spmd

B, H, W, C = 32, 128, 128, 128
KH = KW = 7
PAD = 3
N_CORES = 8
BPC = B // N_CORES  # samples per core
WP = W + 2 * PAD  # 134: x-padded width
CB = 32  # channels per group (pipeline stage)
N_GROUPS = C // CB
CC = 8  # T band-chunk column width
N_CHUNK = 128 // CC
WRLEN = 256  # per-(b,c,j) band buffer length (u = p + y in [0, 254])

_PROGRAM_CACHE = {}


def _chunk_geometry(q, full):
    """Rows [P0, P0+R) of T covering band cells for columns [CC*q, CC*q+CC)."""
    if full:
        return 0, 128
    p0 = max(0, 117 - CC * q)
    pend = min(127, 130 - CC * q)
    return p0, pend - p0 + 1


def _build_program(repeat=1):
    f32 = mybir.dt.float32
    bf16 = mybir.dt.bfloat16
    nc = bacc.Bacc("TRN2", target_bir_lowering=False, debug=False)
    # x: [b, y(flipped), c, x(padded)]; w: band buffers; o: [b, y, c, x]
    x_h = nc.dram_tensor("x", [BPC, H, C, WP], bf16, kind="ExternalInput")
    w_h = nc.dram_tensor(
        "w", [BPC, N_GROUPS, WRLEN, KW, CB], bf16, kind="ExternalInput"
    )
    o_h = nc.dram_tensor("o", [BPC, H, C, W], bf16, kind="ExternalOutput")
    x, o = x_h.ap(), o_h.ap()

    # wr strides (elements): [b, g, u, j, c]
    SW_U = KW * CB
    SW_G = WRLEN * SW_U
    SW_B = N_GROUPS * SW_G

    with tile.TileContext(nc) as tc:
        with (
            tc.tile_pool(name="xbuf", bufs=1) as xpool,
            tc.tile_pool(name="tbuf", bufs=1) as tpool,
            tc.tile_pool(name="obuf", bufs=1) as opool,
            tc.tile_pool(name="psum", bufs=8, space="PSUM") as psump,
        ):
            xb = [xpool.tile([128, CB, WP], bf16, name=f"xb{i}") for i in range(2)]
            # T super-tile, c-minor: [p=y'flip, m=y, j, c]
            tb = [tpool.tile([128, 128, KW, CB], bf16, name=f"tb{i}") for i in range(2)]
            ob = [opool.tile([128, CB, W], bf16, name=f"ob{i}") for i in range(2)]

            groups = [
                (b, g)
                for _ in range(repeat)
                for b in range(BPC)
                for g in range(N_GROUPS)
            ]
            n = len(groups)

            def emit_in_dmas(gi):
                """Input DMAs for group gi (X image batch + T band rewrites)."""
                b, g = groups[gi]
                par = gi % 2
                c0 = g * CB
                xt, tt = xb[par], tb[par]
                # input image batch: [y(128 part), c(CB), x(134)]
                nc.sync.dma_start(out=xt, in_=x[b, :, c0 : c0 + CB, :])
                # band rewrites: one DMA per column-chunk, all (c, j) at once.
                # First use of each buffer writes the full partition range so
                # the static zeros get initialized.
                full = gi < 2
                for q in range(N_CHUNK):
                    p0, r = _chunk_geometry(q, full)
                    dst = tt[p0 : p0 + r, CC * q : CC * q + CC, :, :]
                    src = bass.AP(
                        tensor=w_h,
                        offset=b * SW_B + g * SW_G + (p0 + CC * q) * SW_U,
                        ap=[[SW_U, r], [SW_U, CC], [CB, KW], [1, CB]],
                    )
                    eng = nc.sync if q % 2 == 0 else nc.scalar
                    eng.dma_start(out=dst, in_=src)

            # Software-pipelined emission: group g+2's input DMAs are issued
            # BEFORE group g's out-DMA on the same (in-order) SP sequencer.
            # Otherwise the out-DMA, head-of-line blocked on g's copies,
            # delays g+2's inputs and stalls the PE at each group boundary.
            emit_in_dmas(0)
            if n > 1:
                emit_in_dmas(1)
            for gi, (b, g) in enumerate(groups):
                par = gi % 2
                c0 = g * CB
                xt, tt, ot = xb[par], tb[par], ob[par]

                for ci in range(CB):
                    pt = psump.tile([128, W], f32, name="pt", tag="pt")
                    for j in range(KW):
                        nc.tensor.matmul(
                            out=pt,
                            lhsT=tt[:, :, j, ci],
                            rhs=xt[:, ci, j : j + W],
                            start=(j == 0),
                            stop=(j == KW - 1),
                        )
                    nc.vector.tensor_copy(out=ot[:, ci, :], in_=pt)

                if gi + 2 < n:
                    emit_in_dmas(gi + 2)
                nc.sync.dma_start(out=o[b, :, c0 : c0 + CB, :], in_=ot)

    nc.compile()
    return nc


def _get_program():
    if "nc" not in _PROGRAM_CACHE:
        _PROGRAM_CACHE["nc"] = _build_program()
    return _PROGRAM_CACHE["nc"]


def _prep_inputs(inputs, kernels):
    """Host-side shard + layout transform. Returns per-core input maps."""
    bf16 = ml_dtypes.bfloat16
    # [B,H,W,C] -> [b, y, c, x], y flipped, x padded to 134
    xt = np.zeros((B, H, C, WP), bf16)
    xt[:, :, :, PAD : PAD + W] = np.transpose(inputs[:, ::-1], (0, 1, 3, 2))
    # band buffers: wr[b, g, u, j, cg] = w[b, 130-u, j, g*CB+cg], u in [124,130]
    wr = np.zeros((B, N_GROUPS, WRLEN, KW, CB), bf16)
    # kernels [b, i, j, c] -> [b, u=130-i, j, c] at u slots 124..130
    kswap = np.transpose(kernels[:, ::-1], (0, 1, 2, 3))  # i reversed: i'=6-i
    # u = 130 - i -> for i' = 6-i: u = 124 + i'
    kr = kswap.reshape(B, KH, KW, N_GROUPS, CB)
    wr[:, :, 124:131] = np.transpose(kr, (0, 3, 1, 2, 4))
    in_maps = []
    for k in range(N_CORES):
        sl = slice(k * BPC, (k + 1) * BPC)
        in_maps.append({"x": xt[sl], "w": wr[sl]})
    return in_maps


def _gather_output(results):
    full = np.concatenate([r["o"] for r in results], axis=0)  # [B, y, c, x]
    return np.ascontiguousarray(
        np.transpose(full, (0, 1, 3, 2)).astype(np.float32)
    )


def run_spmd(inputs, kernels, **spmd_kwargs):
    """Run on all 8 cores; returns (output, BassKernelResults)."""
    nc = _get_program()
    in_maps = _prep_inputs(np.asarray(inputs), np.asarray(kernels))
    res = run_bass_kernel_spmd(nc, in_maps, list(range(N_CORES)), **spmd_kwargs)
    return _gather_output(res.results), res


def kernel(inputs, kernels):
    out, _ = run_spmd(inputs, kernels)
    return out
